# revision 55
# baseline (speedup 1.0000x reference)
"""Causal single-head attention (B=4, T=4096, C=1024, H=64) on 8 TRN2 NeuronCores.

Sharding: core = 2*b + p handles batch b and the 16 query/key row-blocks
(128 rows each) of parity p (block-cyclic over T for causal load balance).
All tensors are cast to bf16 on the host (rel err ~4e-3, well inside the
2e-2 gate), halving x DMA traffic and the k/v exchange bytes.

Per 512-column tile tau, each core:
  - projects q/k from its pre-transposed x^T slice (q/k output-stationary
    over heads), and v in [t, h] orientation (32 small matmuls into one
    PSUM bank as a single accumulation group) so no transposes are needed;
  - exchanges packed k/v with its pair peer via AllGather (128KB bf16);
    the readback writes k^T tiles and the [v|1] PV operands directly;
  - attention: S^T = k^T.T @ q^T (bf16, N=512), P^T = exp(S^T/8) on ACT
    (the ~37us of exp work is the roofline engine), causal masking via 0/1
    mask data per parity, then output-stationary PV: out[t,h] += P^T.T @
    [v|1] with free dim 65 (col 64 accumulates the softmax denominator).
    Each tile's pv bank is ONE psum accumulation group (start zeroes the
    whole 2KB zero-region once, stop on the tile's last matmul).
  - normalize with one vectorized reciprocal + broadcast multiply straight
    out of PSUM, store [t,h] partition-major at full DMA rate.

Scheduling: a PE warmup burst completes the p-state ramp before the first
projection; projections run ahead of attention; diagonal pairs run first
in mid tiles (their k/v arrived a tile ago) and last in the final tile to
shorten the tail; DMA issue is spread over SP (x/exchange/stores), ACT
(tile-0 readback) and GPSIMD (collective stand-ins) queues; output stores
are deferred so they never head-block exchange DMAs on the in-order SP
queue.
"""
import numpy as np

import concourse.bacc as bacc
import concourse.bass as bass
import concourse.mybir as mybir
import concourse.tile as tile

dt = mybir.dt
F32R = dt.float32r
F32 = dt.float32
BF16 = dt.bfloat16

B, T, C, H = 4, 4096, 1024, 64
NBLK = T // 128            # 32 global blocks per batch
NLOC = NBLK // 2           # 16 blocks per core
NT = NLOC * 128            # 2048 query rows per core
NTT = NT // 512            # 4 t-tiles per core
N_CORES = 8
GROUPS = [[0, 1], [2, 3], [4, 5], [6, 7]]
SCALE = 1.0 / np.sqrt(H)

EXP = mybir.ActivationFunctionType.Exp


def _emit_body(nc, tc, aps, pools, rep):
    (xT_ap, wqkv_ap, cst_ap, out_ap) = aps
    sb, ps, dr = pools

    # --- constants in SBUF ---
    wqkv = sb.tile([128, 8 * 192], BF16, tag="wqkv", name=f"wqkv{rep}")
    cst = sb.tile([128, 256], BF16, tag="cst", name=f"cst{rep}")
    masks = cst[:, 0:256]
    nc.sync.dma_start(wqkv[:], wqkv_ap[:])

    # --- persistent activations (bf16) ---
    qT_sb = sb.tile([64, NT], BF16, tag="qT", name=f"qT{rep}")
    kT_par = [sb.tile([64, NT], BF16, tag=f"kTp{j}", name=f"kTp{j}_{rep}")
              for j in (0, 1)]
    # vaug[j][:, 65*i : 65*i+65] = [v block | ones] for global block s=2i+j
    vaug = [sb.tile([128, NLOC * 65], BF16, tag=f"vaug{j}", name=f"vaug{j}_{rep}")
            for j in (0, 1)]

    xT_3d = xT_ap[:].rearrange("(g p) n -> p g n", p=128)          # [128, 8, NT]
    xts = {}

    # PE warmup: ~3.5us of dummy matmuls so the p-state ramp completes
    # before the first projection matmul.
    warm = sb.tile([64, 576], BF16, tag="warm", name=f"warm{rep}")
    nc.gpsimd.memset(warm[:], 1.0)
    for j in (0, 1):
        nc.gpsimd.memset(
            vaug[j][:].rearrange("p (s n) -> p s n", n=65)[:, :, 64:65], 1.0)
    wsp = ps.tile([128, 1024], F32, tag="sp", bufs=2, name=f"warmsp{rep}")
    for i in range(8):
        nc.tensor.matmul(wsp[0:64, 0:512], warm[:, 0:64], warm[:, 64:576],
                         start=True, stop=True)

    def emit_xt_load(tau):
        t0 = 512 * tau
        xt = sb.tile([128, 8 * 512], BF16, tag="xt", bufs=4, name=f"xt{rep}_{tau}")
        xts[tau] = xt
        half = xt[:].rearrange("p (g n) -> p g n", g=8)
        if tau == 0:
            nc.sync.dma_start(half[:, 0:2], xT_3d[:, 0:2, t0:t0+512])
            nc.sync.dma_start(half[:, 2:5], xT_3d[:, 2:5, t0:t0+512])
            nc.sync.dma_start(half[:, 5:8], xT_3d[:, 5:8, t0:t0+512])
            nc.sync.dma_start(cst[:], cst_ap[:])
        elif tau == 1:
            for g in range(0, 8, 2):
                nc.sync.dma_start(half[:, g:g+2], xT_3d[:, g:g+2, t0:t0+512])
        else:
            nc.sync.dma_start(half[:, 0:4], xT_3d[:, 0:4, t0:t0+512])
            nc.sync.dma_start(half[:, 4:8], xT_3d[:, 4:8, t0:t0+512])

    qkps = {}

    def emit_proj_mms(tau):
        """Projection matmuls as a list of thunks (fill work between pairs)."""
        xt = xts[tau]
        qkp = ps.tile([128, 512], F32, tag="qkp", name=f"qkp{rep}_{tau}")
        # v projected output-stationary: [128 t, 4*64 h] — one bank, one
        # accumulation group (start zeroes the whole 2KB zero-region once)
        vp = ps.tile([128, 256], F32, tag="vp", name=f"vp{rep}_{tau}")
        qkps[tau] = (qkp, vp)
        thunks = []
        for c in range(8):
            thunks.append(lambda c=c: nc.tensor.matmul(
                qkp[:], wqkv[:, 192*c:192*c+128], xt[:, 512*c:512*(c+1)],
                start=(c == 0), stop=(c == 7)))
        for c in range(8):
            def vmm(c=c):
                for tb in range(4):
                    nc.tensor.matmul(
                        vp[:, 64*tb:64*(tb+1)],
                        xt[:, 512*c+128*tb:512*c+128*(tb+1)],
                        wqkv[:, 192*c+128:192*(c+1)],
                        start=(c == 0 and tb == 0), stop=(c == 7 and tb == 3))
            thunks.append(vmm)
        return thunks

    def emit_proj_rest(tau):
        t0 = 512 * tau
        qkp, vp = qkps[tau]
        # kv layout: cols 0:256 = k^T split over both partition halves,
        # cols 256:512 = v blocks in [t, h] orientation
        kv = sb.tile([128, 512], BF16, tag="kv", bufs=4, name=f"kv{rep}_{tau}")
        nc.vector.tensor_copy(kv[0:64, 0:256], qkp[64:128, 0:256])
        nc.vector.tensor_copy(kv[64:128, 0:256], qkp[64:128, 256:512])
        nc.vector.tensor_copy(kv[:, 256:512], vp[:])
        nc.vector.tensor_copy(qT_sb[:, t0:t0+512], qkp[0:64, :])

        ccin = dr.tile([128, 512], BF16, tag="ccin", bufs=4, name=f"ccin{rep}_{tau}")  # noqa
        ccout = dr.tile([2, 128, 512], BF16, tag="ccout", bufs=4, name=f"ccout{rep}_{tau}")
        with tc.high_priority():
            nc.sync.dma_start(ccin[:], kv[:])
            if nc.num_devices > 1:
                nc.gpsimd.collective_compute(
                    "AllGather", mybir.AluOpType.bypass, replica_groups=GROUPS,
                    ins=[ccin[:]], outs=[ccout[:]],
                )
                for j in (0, 1):
                    nc.sync.dma_start(
                        kT_par[j][:, t0:t0+512].rearrange("p (h n) -> p h n", h=2),
                        ccout[j, :, 0:256].rearrange("(h p) n -> p h n", p=64))
                for j in (0, 1):
                    nc.sync.dma_start(
                        vaug[j][:, 260*tau:260*(tau+1)].rearrange(
                            "p (s n) -> p s n", n=65)[:, :, 0:64],
                        ccout[j, :, 256:512].rearrange("p (s n) -> p s n", n=64))
            else:
                # single-core timing sim: stand-in DMAs with the collective's
                # traffic; k/v readback sources from ccin (identical bytes)
                nc.gpsimd.dma_start(ccout[0], ccin[:])
                nc.gpsimd.dma_start(ccout[1], ccin[:])
                read_eng = nc.scalar if tau == 0 else nc.sync
                for j in (0, 1):
                    read_eng.dma_start(
                        kT_par[j][:, t0:t0+512].rearrange("p (h n) -> p h n", h=2),
                        ccin[:, 0:256].rearrange("(h p) n -> p h n", p=64))
                for j in (0, 1):
                    read_eng.dma_start(
                        vaug[j][:, 260*tau:260*(tau+1)].rearrange(
                            "p (s n) -> p s n", n=65)[:, :, 0:64],
                        ccin[:, 256:512].rearrange("p (s n) -> p s n", n=64))

    pv_bufs = [ps.tile([128, 512], F32, tag="pv", bufs=2, name=f"pvb{rep}_{i}")
               for i in range(2)]
    pvs = {}

    def emit_pair(tau, m, is_first, is_last):
        """One attention pair (key blocks 2m, 2m+1) for t-tile tau.
        is_first/is_last: emission-order first/last pair of the tile.  The
        whole pv bank is one psum accumulation group: start=True (which
        zeroes the full 2KB zero-region) only on the tile's first PV matmul,
        stop=True only on its last."""
        t0 = 512 * tau
        pv = pvs[tau]
        d0 = 2 * m - 8 * tau
        k = d0 // 2 if d0 >= 0 else 0
        off = 128 * k
        sp = ps.tile([128, 1024], F32, tag="sp", bufs=2, name=f"sp{rep}_{tau}_{m}")
        for idx in (0, 1):
            nc.tensor.matmul(
                sp[:, 512*idx+off:512*(idx+1)],
                kT_par[idx][:, 128*m:128*(m+1)],
                qT_sb[:, t0+off:t0+512],
                start=True, stop=True)
        pt = sb.tile([128, 1024], BF16, tag="pt", bufs=8, name=f"pt{rep}_{tau}_{m}")
        sp_seg = sp[:].rearrange("p (s n) -> p s n", s=2)[:, :, off:512]
        pt_seg = pt[:].rearrange("p (s n) -> p s n", s=2)[:, :, off:512]
        nc.scalar.activation(pt_seg, sp_seg, EXP, scale=float(SCALE))
        if d0 >= 0:
            for idx in (0, 1):
                seg = pt[:, 512*idx+off:512*idx+off+128]
                nc.vector.tensor_mul(seg, seg, masks[:, 128*idx:128*(idx+1)])
        for idx in (0, 1):
            for j in range(k, 4):
                nc.tensor.matmul(
                    pv[:, 65*j:65*(j+1)],
                    pt[:, 512*idx+128*j:512*idx+128*(j+1)],
                    vaug[idx][:, 65*m:65*(m+1)],
                    start=(is_first and idx == 0 and j == k),
                    stop=(is_last and idx == 1 and j == 3))

    def emit_finalize(tau, ofin):
        pv = pvs[tau]
        pv3 = pv[:, 0:260].rearrange("p (j n) -> p j n", n=65)
        rc = sb.tile([128, 4], F32, tag="rc", bufs=2, name=f"rc{rep}_{tau}")
        nc.vector.reciprocal(rc[:], pv3[:, :, 64:65])
        rcb = rc[:].rearrange("p (j o) -> p j o", o=1).broadcast_to([128, 4, 64])
        nc.vector.tensor_mul(ofin[:].rearrange("p (j h) -> p j h", h=64),
                             pv3[:, :, 0:64], rcb)

    out_stores = []

    def emit_attn(tau):
        """All pairs for t-tile tau.  For tau >= 1 the diagonal pairs run
        first (their k/v arrived a tile ago), so the tile ends on cheap
        off-diagonal pairs rather than a serial diagonal chain."""
        pvs[tau] = pv_bufs[tau % 2]
        ofin = sb.tile([128, 4 * 64], F32, tag="ofin", bufs=4, name=f"of{rep}_{tau}")
        n_early = 4 * tau
        diag_last = tau in (1, NTT - 1) and n_early > 0
        if diag_last:                      # last tile ends on the small
            for m in range(n_early):       # diagonal pairs for a short tail
                emit_pair(tau, m, m == 0, False)
            for j in range(4):
                emit_pair(tau, 4 * tau + j, False, j == 3)
        else:
            for j in range(4):
                emit_pair(tau, 4 * tau + j, j == 0, j == 3 and n_early == 0)
            for m in range(n_early):       # early (off-diagonal) pairs
                emit_pair(tau, m, False, m == n_early - 1)
        emit_finalize(tau, ofin)
        # stores are deferred so they never head-block exchange DMAs
        out_stores.append(
            lambda tau=tau, ofin=ofin:
                nc.sync.dma_start(out_ap[:, 256*tau:256*(tau+1)], ofin[:]))

    def emit_proj(tau):
        for t in emit_proj_mms(tau):
            t()
        emit_proj_rest(tau)

    # ---- software pipeline: attention follows its own tile's projection;
    # the next projection interleaves behind it on the in-order queues
    emit_xt_load(0)
    emit_xt_load(1)
    emit_proj(0)
    emit_xt_load(2)
    emit_attn(0)
    emit_proj(1)
    emit_xt_load(3)
    emit_proj(2)
    emit_attn(1)
    emit_attn(2)
    emit_proj(3)
    for st in out_stores[:3]:
        st()
    emit_attn(3)
    for st in out_stores[3:]:
        st()


def build(reps=1, n_devices=N_CORES):
    nc = bacc.Bacc("TRN2", target_bir_lowering=False, debug=False,
                   num_devices=n_devices)
    xT_ap = nc.dram_tensor("xT", [C, NT], BF16, kind="ExternalInput").ap()
    wqkv_ap = nc.dram_tensor("wqkv", [128, 8 * 192], BF16, kind="ExternalInput").ap()
    cst_ap = nc.dram_tensor("cst", [128, 256], BF16, kind="ExternalInput").ap()
    out_ap = nc.dram_tensor("out", [128, NLOC * H], F32, kind="ExternalOutput").ap()
    aps = (xT_ap, wqkv_ap, cst_ap, out_ap)

    with tile.TileContext(nc) as tc:
        with tc.tile_pool(name="sb", bufs=1) as sb, \
             tc.tile_pool(name="ps", bufs=1, space="PSUM") as ps, \
             tc.tile_pool(name="dr", bufs=1, space="DRAM") as dr:
            for rep in range(reps):
                _emit_body(nc, tc, aps, (sb, ps, dr), rep)
    nc.compile()
    return nc


def make_inputs(x, Wq, Wk, Wv):
    """Per-core input maps from full inputs."""
    x = np.asarray(x, dtype=np.float32)
    Wq, Wk, Wv = (np.asarray(w, dtype=np.float32) for w in (Wq, Wk, Wv))
    bf16 = mybir.dt.np(BF16)
    wqkv = np.concatenate([Wq, Wk, Wv], axis=1)                 # [C, 192]
    wqkv = np.ascontiguousarray(
        wqkv.reshape(8, 128, 192).transpose(1, 0, 2).reshape(128, 8 * 192)
    ).astype(bf16)
    tri = (np.arange(128)[:, None] <= np.arange(128)[None, :]).astype(np.float32)
    zeros = np.zeros((128, 128), np.float32)
    ones = np.ones((128, 128), np.float32)

    in_maps = []
    for core in range(N_CORES):
        b, p = core // 2, core % 2
        xT = np.ascontiguousarray(
            x[b].T.reshape(C, NBLK, 128)[:, p::2, :].reshape(C, NT)).astype(bf16)
        cst = np.concatenate(
            [tri, zeros] if p == 0 else [ones, tri], axis=1).astype(bf16)
        in_maps.append({"xT": xT, "wqkv": wqkv, "cst": cst})
    return in_maps


def gather_output(results):
    """results: list per core of {"out": [128, NLOC*H]} → [B, T, H]."""
    out = np.empty((B, T, H), dtype=np.float32)
    for core in range(N_CORES):
        b, p = core // 2, core % 2
        o = results[core]["out"].reshape(128, NLOC, H).transpose(1, 0, 2)
        out[b].reshape(NBLK, 128, H)[p::2] = o
    return out


# ---------------------------------------------------------------------------
# held PJRT runner (axon path) — inlined so kernel.py is self-contained
# ---------------------------------------------------------------------------

def make_runner(nc, n_cores):
    import jax
    from jax.sharding import Mesh, PartitionSpec
    from jax.experimental.shard_map import shard_map
    from concourse import bass2jax
    from concourse.bass2jax import _bass_exec_p, install_neuronx_cc_hook

    install_neuronx_cc_hook()
    partition_name = nc.partition_id_tensor.name if nc.partition_id_tensor else None

    in_names, out_names, out_avals, zero_shapes = [], [], [], []
    for alloc in nc.m.functions[0].allocations:
        if not isinstance(alloc, mybir.MemoryLocationSet):
            continue
        name = alloc.memorylocations[0].name
        if alloc.kind == "ExternalInput":
            if name != partition_name:
                in_names.append(name)
        elif alloc.kind == "ExternalOutput":
            out_names.append(name)
            shape = tuple(alloc.tensor_shape)
            dtype = mybir.dt.np(alloc.dtype)
            out_avals.append(jax.core.ShapedArray(shape, dtype))
            zero_shapes.append((shape, dtype))
    n_params, n_outs = len(in_names), len(out_avals)
    all_in_names = list(in_names) + list(out_names)
    if partition_name is not None:
        all_in_names.append(partition_name)
    donate = tuple(range(n_params, n_params + n_outs))

    def _body(*args):
        operands = list(args)
        if partition_name is not None:
            operands.append(bass2jax.partition_id_tensor())
        outs = _bass_exec_p.bind(
            *operands, out_avals=tuple(out_avals), in_names=tuple(all_in_names),
            out_names=tuple(out_names), lowering_input_output_aliases=(),
            sim_require_finite=True, sim_require_nnan=True, nc=nc)
        return tuple(outs)

    devices = jax.devices()[:n_cores]
    mesh = Mesh(np.asarray(devices), ("core",))
    sharded = jax.jit(
        shard_map(_body, mesh=mesh,
                  in_specs=(PartitionSpec("core"),) * (n_params + n_outs),
                  out_specs=(PartitionSpec("core"),) * n_outs, check_rep=False),
        donate_argnums=donate, keep_unused=True)
    make_zeros = jax.jit(lambda: tuple(
        jax.numpy.zeros((n_cores * s[0], *s[1:]), d) for (s, d) in zero_shapes))

    class Runner:
        def commit_inputs(self, in_maps):
            per_core = [[np.asarray(m[name]) for name in in_names] for m in in_maps]
            concat = [np.concatenate([per_core[c][i] for c in range(n_cores)], axis=0)
                      for i in range(n_params)]
            self._committed = [jax.device_put(a) for a in concat]
            jax.block_until_ready(self._committed)

        def run(self):
            outs = sharded(*self._committed, *make_zeros())
            jax.block_until_ready(outs)
            return outs

        def results(self, outs):
            res = [dict() for _ in range(n_cores)]
            for i, name in enumerate(out_names):
                per = np.split(np.asarray(outs[i]), n_cores, axis=0)
                for c in range(n_cores):
                    res[c][name] = per[c]
            return res

    return Runner()


_cache = {}


def get_runner(reps=1):
    if reps not in _cache:
        nc = build(reps)
        _cache[reps] = make_runner(nc, N_CORES)
    return _cache[reps]


def kernel(x, Wq, Wk, Wv):
    r = get_runner(1)
    r.commit_inputs(make_inputs(x, Wq, Wk, Wv))
    return gather_output(r.results(r.run()))


# revision 73
# speedup vs baseline: 1.0023x; 1.0023x over previous
"""Causal single-head attention (B=4, T=4096, C=1024, H=64) on 8 TRN2 NeuronCores.

Sharding: core = 2*b + p handles batch b and the 16 query/key row-blocks
(128 rows each) of parity p (block-cyclic over T for causal load balance).
All tensors are cast to bf16 on the host (rel err ~4e-3, well inside the
2e-2 gate), halving x DMA traffic and the k/v exchange bytes.

Per 512-column tile tau, each core:
  - projects q/k from its pre-transposed x^T slice (q/k output-stationary
    over heads), and v in [t, h] orientation (32 small matmuls into one
    PSUM bank as a single accumulation group) so no transposes are needed;
  - exchanges packed k/v with its pair peer via AllGather (128KB bf16);
    the readback writes k^T tiles and the [v|1] PV operands directly;
  - attention: S^T = k^T.T @ q^T (bf16, N=512), P^T = exp(S^T/8) on ACT
    (the ~37us of exp work is the roofline engine), causal masking via 0/1
    mask data per parity, then output-stationary PV: out[t,h] += P^T.T @
    [v|1] with free dim 65 (col 64 accumulates the softmax denominator).
    Each tile's pv bank is ONE psum accumulation group (start zeroes the
    whole 2KB zero-region once, stop on the tile's last matmul).
  - normalize with one vectorized reciprocal + broadcast multiply straight
    out of PSUM, store [t,h] partition-major at full DMA rate.

Scheduling: a PE warmup burst completes the p-state ramp before the first
projection; projections run ahead of attention; diagonal pairs run first
in mid tiles (their k/v arrived a tile ago) and last in the final tile to
shorten the tail; DMA issue is spread over SP (x/exchange/stores), ACT
(tile-0 readback) and GPSIMD (collective stand-ins) queues; output stores
are deferred so they never head-block exchange DMAs on the in-order SP
queue.
"""
import numpy as np

import concourse.bacc as bacc
import concourse.bass as bass
import concourse.mybir as mybir
import concourse.tile as tile

dt = mybir.dt
F32R = dt.float32r
F32 = dt.float32
BF16 = dt.bfloat16

B, T, C, H = 4, 4096, 1024, 64
NBLK = T // 128            # 32 global blocks per batch
NLOC = NBLK // 2           # 16 blocks per core
NT = NLOC * 128            # 2048 query rows per core
NTT = NT // 512            # 4 t-tiles per core
N_CORES = 8
GROUPS = [[0, 1], [2, 3], [4, 5], [6, 7]]
SCALE = 1.0 / np.sqrt(H)

EXP = mybir.ActivationFunctionType.Exp


def _emit_body(nc, tc, aps, pools, rep):
    (xT_ap, wqkv_ap, cst_ap, out_ap) = aps
    sb, ps, dr = pools

    # --- constants in SBUF ---
    wqkv = sb.tile([128, 8 * 192], BF16, tag="wqkv", name=f"wqkv{rep}")
    cst = sb.tile([128, 256], BF16, tag="cst", name=f"cst{rep}")
    masks = cst[:, 0:256]
    nc.sync.dma_start(wqkv[:], wqkv_ap[:])

    # --- persistent activations (bf16) ---
    qT_sb = sb.tile([64, NT], BF16, tag="qT", name=f"qT{rep}")
    kT_par = [sb.tile([64, NT], BF16, tag=f"kTp{j}", name=f"kTp{j}_{rep}")
              for j in (0, 1)]
    # vaug[j][:, 65*i : 65*i+65] = [v block | ones] for global block s=2i+j
    vaug = [sb.tile([128, NLOC * 65], BF16, tag=f"vaug{j}", name=f"vaug{j}_{rep}")
            for j in (0, 1)]

    xT_3d = xT_ap[:].rearrange("(g p) n -> p g n", p=128)          # [128, 8, NT]
    xts = {}

    # PE warmup: ~3.5us of dummy matmuls so the p-state ramp completes
    # before the first projection matmul.
    warm = sb.tile([64, 576], BF16, tag="warm", name=f"warm{rep}")
    nc.gpsimd.memset(warm[:], 1.0)
    for j in (0, 1):
        nc.gpsimd.memset(
            vaug[j][:].rearrange("p (s n) -> p s n", n=65)[:, :, 64:65], 1.0)
    wsp = ps.tile([128, 1024], F32, tag="sp", bufs=2, name=f"warmsp{rep}")
    for i in range(8):
        nc.tensor.matmul(wsp[0:64, 0:512], warm[:, 0:64], warm[:, 64:576],
                         start=True, stop=True)

    def emit_xt_load(tau):
        t0 = 512 * tau
        xt = sb.tile([128, 8 * 512], BF16, tag="xt", bufs=4, name=f"xt{rep}_{tau}")
        xts[tau] = xt
        half = xt[:].rearrange("p (g n) -> p g n", g=8)
        if tau == 0:
            nc.sync.dma_start(half[:, 0:2], xT_3d[:, 0:2, t0:t0+512])
            nc.sync.dma_start(half[:, 2:5], xT_3d[:, 2:5, t0:t0+512])
            nc.sync.dma_start(half[:, 5:8], xT_3d[:, 5:8, t0:t0+512])
            nc.sync.dma_start(cst[:], cst_ap[:])
        elif tau == 1:
            for g in range(0, 8, 2):
                nc.sync.dma_start(half[:, g:g+2], xT_3d[:, g:g+2, t0:t0+512])
        else:
            nc.sync.dma_start(half[:, 0:4], xT_3d[:, 0:4, t0:t0+512])
            nc.sync.dma_start(half[:, 4:8], xT_3d[:, 4:8, t0:t0+512])

    qkps = {}

    def emit_proj_mms(tau):
        """Projection matmuls as a list of thunks (fill work between pairs)."""
        xt = xts[tau]
        qkp = ps.tile([128, 512], F32, tag="qkp", name=f"qkp{rep}_{tau}")
        # v projected output-stationary: [128 t, 4*64 h] — one bank, one
        # accumulation group (start zeroes the whole 2KB zero-region once)
        vp = ps.tile([128, 256], F32, tag="vp", name=f"vp{rep}_{tau}")
        qkps[tau] = (qkp, vp)
        thunks = []
        for c in range(8):
            thunks.append(lambda c=c: nc.tensor.matmul(
                qkp[:], wqkv[:, 192*c:192*c+128], xt[:, 512*c:512*(c+1)],
                start=(c == 0), stop=(c == 7)))
        for c in range(8):
            def vmm(c=c):
                for tb in range(4):
                    nc.tensor.matmul(
                        vp[:, 64*tb:64*(tb+1)],
                        xt[:, 512*c+128*tb:512*c+128*(tb+1)],
                        wqkv[:, 192*c+128:192*(c+1)],
                        start=(c == 0 and tb == 0), stop=(c == 7 and tb == 3))
            thunks.append(vmm)
        return thunks

    def emit_proj_rest(tau):
        t0 = 512 * tau
        qkp, vp = qkps[tau]
        # kv layout: cols 0:256 = k^T split over both partition halves,
        # cols 256:512 = v blocks in [t, h] orientation
        kv = sb.tile([128, 512], BF16, tag="kv", bufs=4, name=f"kv{rep}_{tau}")
        nc.vector.tensor_copy(kv[0:64, 0:256], qkp[64:128, 0:256])
        nc.vector.tensor_copy(kv[64:128, 0:256], qkp[64:128, 256:512])
        nc.vector.tensor_copy(kv[:, 256:512], vp[:])
        nc.vector.tensor_copy(qT_sb[:, t0:t0+512], qkp[0:64, :])

        ccin = dr.tile([128, 512], BF16, tag="ccin", bufs=4, name=f"ccin{rep}_{tau}")  # noqa
        ccout = dr.tile([2, 128, 512], BF16, tag="ccout", bufs=4, name=f"ccout{rep}_{tau}")
        with tc.high_priority():
            nc.sync.dma_start(ccin[:], kv[:])
            if nc.num_devices > 1:
                nc.gpsimd.collective_compute(
                    "AllGather", mybir.AluOpType.bypass, replica_groups=GROUPS,
                    ins=[ccin[:]], outs=[ccout[:]],
                )
                for j in (0, 1):
                    nc.sync.dma_start(
                        kT_par[j][:, t0:t0+512].rearrange("p (h n) -> p h n", h=2),
                        ccout[j, :, 0:256].rearrange("(h p) n -> p h n", p=64))
                for j in (0, 1):
                    nc.sync.dma_start(
                        vaug[j][:, 260*tau:260*(tau+1)].rearrange(
                            "p (s n) -> p s n", n=65)[:, :, 0:64],
                        ccout[j, :, 256:512].rearrange("p (s n) -> p s n", n=64))
            else:
                # single-core timing sim: stand-in DMAs with the collective's
                # traffic; k/v readback sources from ccin (identical bytes)
                nc.gpsimd.dma_start(ccout[0], ccin[:])
                nc.gpsimd.dma_start(ccout[1], ccin[:])
                read_eng = nc.scalar if tau == 0 else nc.sync
                for j in (0, 1):
                    read_eng.dma_start(
                        kT_par[j][:, t0:t0+512].rearrange("p (h n) -> p h n", h=2),
                        ccin[:, 0:256].rearrange("(h p) n -> p h n", p=64))
                for j in (0, 1):
                    read_eng.dma_start(
                        vaug[j][:, 260*tau:260*(tau+1)].rearrange(
                            "p (s n) -> p s n", n=65)[:, :, 0:64],
                        ccin[:, 256:512].rearrange("p (s n) -> p s n", n=64))

    pv_bufs = [ps.tile([128, 512], F32, tag="pv", bufs=2, name=f"pvb{rep}_{i}")
               for i in range(2)]
    pvs = {}

    def emit_pair(tau, m, is_first, is_last):
        """One attention pair (key blocks 2m, 2m+1) for t-tile tau.
        is_first/is_last: emission-order first/last pair of the tile.  The
        whole pv bank is one psum accumulation group: start=True (which
        zeroes the full 2KB zero-region) only on the tile's first PV matmul,
        stop=True only on its last."""
        t0 = 512 * tau
        pv = pvs[tau]
        d0 = 2 * m - 8 * tau
        k = d0 // 2 if d0 >= 0 else 0
        off = 128 * k
        sp = ps.tile([128, 1024], F32, tag="sp", bufs=2, name=f"sp{rep}_{tau}_{m}")
        for idx in (0, 1):
            nc.tensor.matmul(
                sp[:, 512*idx+off:512*(idx+1)],
                kT_par[idx][:, 128*m:128*(m+1)],
                qT_sb[:, t0+off:t0+512],
                start=True, stop=True)
        pt = sb.tile([128, 1024], BF16, tag="pt", bufs=8, name=f"pt{rep}_{tau}_{m}")
        sp_seg = sp[:].rearrange("p (s n) -> p s n", s=2)[:, :, off:512]
        pt_seg = pt[:].rearrange("p (s n) -> p s n", s=2)[:, :, off:512]
        nc.scalar.activation(pt_seg, sp_seg, EXP, scale=float(SCALE))
        if d0 >= 0:
            seg = pt[:].rearrange("p (s n) -> p s n", s=2)[:, :, off:off+128]
            nc.vector.tensor_mul(
                seg, seg, masks[:].rearrange("p (s n) -> p s n", s=2))
        for idx in (0, 1):
            for j in range(k, 4):
                nc.tensor.matmul(
                    pv[:, 65*j:65*(j+1)],
                    pt[:, 512*idx+128*j:512*idx+128*(j+1)],
                    vaug[idx][:, 65*m:65*(m+1)],
                    start=(is_first and idx == 0 and j == k),
                    stop=(is_last and idx == 1 and j == 3))

    def emit_finalize(tau, ofin):
        pv = pvs[tau]
        pv3 = pv[:, 0:260].rearrange("p (j n) -> p j n", n=65)
        rc = sb.tile([128, 4], F32, tag="rc", bufs=2, name=f"rc{rep}_{tau}")
        nc.vector.reciprocal(rc[:], pv3[:, :, 64:65])
        rcb = rc[:].rearrange("p (j o) -> p j o", o=1).broadcast_to([128, 4, 64])
        nc.vector.tensor_mul(ofin[:].rearrange("p (j h) -> p j h", h=64),
                             pv3[:, :, 0:64], rcb)

    out_stores = []

    def emit_attn(tau):
        """All pairs for t-tile tau.  For tau >= 1 the diagonal pairs run
        first (their k/v arrived a tile ago), so the tile ends on cheap
        off-diagonal pairs rather than a serial diagonal chain."""
        pvs[tau] = pv_bufs[tau % 2]
        ofin = sb.tile([128, 4 * 64], F32, tag="ofin", bufs=4, name=f"of{rep}_{tau}")
        n_early = 4 * tau
        diag_last = tau in (1, NTT - 1) and n_early > 0
        if diag_last:                      # last tile ends on the small
            for m in range(n_early):       # diagonal pairs for a short tail
                emit_pair(tau, m, m == 0, False)
            for j in range(4):
                emit_pair(tau, 4 * tau + j, False, j == 3)
        else:
            for j in range(4):
                emit_pair(tau, 4 * tau + j, j == 0, j == 3 and n_early == 0)
            for m in range(n_early):       # early (off-diagonal) pairs
                emit_pair(tau, m, False, m == n_early - 1)
        emit_finalize(tau, ofin)
        # stores are deferred so they never head-block exchange DMAs
        out_stores.append(
            lambda tau=tau, ofin=ofin:
                nc.sync.dma_start(out_ap[:, 256*tau:256*(tau+1)], ofin[:]))

    def emit_proj(tau):
        for t in emit_proj_mms(tau):
            t()
        emit_proj_rest(tau)

    # ---- software pipeline: attention follows its own tile's projection;
    # the next projection interleaves behind it on the in-order queues
    emit_xt_load(0)
    emit_xt_load(1)
    emit_proj(0)
    emit_xt_load(2)
    emit_attn(0)
    emit_proj(1)
    emit_xt_load(3)
    emit_proj(2)
    emit_attn(1)
    emit_attn(2)
    emit_proj(3)
    for st in out_stores[:3]:
        st()
    emit_attn(3)
    for st in out_stores[3:]:
        st()


def build(reps=1, n_devices=N_CORES):
    nc = bacc.Bacc("TRN2", target_bir_lowering=False, debug=False,
                   num_devices=n_devices)
    xT_ap = nc.dram_tensor("xT", [C, NT], BF16, kind="ExternalInput").ap()
    wqkv_ap = nc.dram_tensor("wqkv", [128, 8 * 192], BF16, kind="ExternalInput").ap()
    cst_ap = nc.dram_tensor("cst", [128, 256], BF16, kind="ExternalInput").ap()
    out_ap = nc.dram_tensor("out", [128, NLOC * H], F32, kind="ExternalOutput").ap()
    aps = (xT_ap, wqkv_ap, cst_ap, out_ap)

    with tile.TileContext(nc) as tc:
        with tc.tile_pool(name="sb", bufs=1) as sb, \
             tc.tile_pool(name="ps", bufs=1, space="PSUM") as ps, \
             tc.tile_pool(name="dr", bufs=1, space="DRAM") as dr:
            for rep in range(reps):
                _emit_body(nc, tc, aps, (sb, ps, dr), rep)
    nc.compile()
    return nc


def make_inputs(x, Wq, Wk, Wv):
    """Per-core input maps from full inputs."""
    x = np.asarray(x, dtype=np.float32)
    Wq, Wk, Wv = (np.asarray(w, dtype=np.float32) for w in (Wq, Wk, Wv))
    bf16 = mybir.dt.np(BF16)
    wqkv = np.concatenate([Wq, Wk, Wv], axis=1)                 # [C, 192]
    wqkv = np.ascontiguousarray(
        wqkv.reshape(8, 128, 192).transpose(1, 0, 2).reshape(128, 8 * 192)
    ).astype(bf16)
    tri = (np.arange(128)[:, None] <= np.arange(128)[None, :]).astype(np.float32)
    zeros = np.zeros((128, 128), np.float32)
    ones = np.ones((128, 128), np.float32)

    in_maps = []
    for core in range(N_CORES):
        b, p = core // 2, core % 2
        xT = np.ascontiguousarray(
            x[b].T.reshape(C, NBLK, 128)[:, p::2, :].reshape(C, NT)).astype(bf16)
        cst = np.concatenate(
            [tri, zeros] if p == 0 else [ones, tri], axis=1).astype(bf16)
        in_maps.append({"xT": xT, "wqkv": wqkv, "cst": cst})
    return in_maps


def gather_output(results):
    """results: list per core of {"out": [128, NLOC*H]} → [B, T, H]."""
    out = np.empty((B, T, H), dtype=np.float32)
    for core in range(N_CORES):
        b, p = core // 2, core % 2
        o = results[core]["out"].reshape(128, NLOC, H).transpose(1, 0, 2)
        out[b].reshape(NBLK, 128, H)[p::2] = o
    return out


# ---------------------------------------------------------------------------
# held PJRT runner (axon path) — inlined so kernel.py is self-contained
# ---------------------------------------------------------------------------

def make_runner(nc, n_cores):
    import jax
    from jax.sharding import Mesh, PartitionSpec
    from jax.experimental.shard_map import shard_map
    from concourse import bass2jax
    from concourse.bass2jax import _bass_exec_p, install_neuronx_cc_hook

    install_neuronx_cc_hook()
    partition_name = nc.partition_id_tensor.name if nc.partition_id_tensor else None

    in_names, out_names, out_avals, zero_shapes = [], [], [], []
    for alloc in nc.m.functions[0].allocations:
        if not isinstance(alloc, mybir.MemoryLocationSet):
            continue
        name = alloc.memorylocations[0].name
        if alloc.kind == "ExternalInput":
            if name != partition_name:
                in_names.append(name)
        elif alloc.kind == "ExternalOutput":
            out_names.append(name)
            shape = tuple(alloc.tensor_shape)
            dtype = mybir.dt.np(alloc.dtype)
            out_avals.append(jax.core.ShapedArray(shape, dtype))
            zero_shapes.append((shape, dtype))
    n_params, n_outs = len(in_names), len(out_avals)
    all_in_names = list(in_names) + list(out_names)
    if partition_name is not None:
        all_in_names.append(partition_name)
    donate = tuple(range(n_params, n_params + n_outs))

    def _body(*args):
        operands = list(args)
        if partition_name is not None:
            operands.append(bass2jax.partition_id_tensor())
        outs = _bass_exec_p.bind(
            *operands, out_avals=tuple(out_avals), in_names=tuple(all_in_names),
            out_names=tuple(out_names), lowering_input_output_aliases=(),
            sim_require_finite=True, sim_require_nnan=True, nc=nc)
        return tuple(outs)

    devices = jax.devices()[:n_cores]
    mesh = Mesh(np.asarray(devices), ("core",))
    sharded = jax.jit(
        shard_map(_body, mesh=mesh,
                  in_specs=(PartitionSpec("core"),) * (n_params + n_outs),
                  out_specs=(PartitionSpec("core"),) * n_outs, check_rep=False),
        donate_argnums=donate, keep_unused=True)
    make_zeros = jax.jit(lambda: tuple(
        jax.numpy.zeros((n_cores * s[0], *s[1:]), d) for (s, d) in zero_shapes))

    class Runner:
        def commit_inputs(self, in_maps):
            per_core = [[np.asarray(m[name]) for name in in_names] for m in in_maps]
            concat = [np.concatenate([per_core[c][i] for c in range(n_cores)], axis=0)
                      for i in range(n_params)]
            self._committed = [jax.device_put(a) for a in concat]
            jax.block_until_ready(self._committed)

        def run(self):
            outs = sharded(*self._committed, *make_zeros())
            jax.block_until_ready(outs)
            return outs

        def results(self, outs):
            res = [dict() for _ in range(n_cores)]
            for i, name in enumerate(out_names):
                per = np.split(np.asarray(outs[i]), n_cores, axis=0)
                for c in range(n_cores):
                    res[c][name] = per[c]
            return res

    return Runner()


_cache = {}


def get_runner(reps=1):
    if reps not in _cache:
        nc = build(reps)
        _cache[reps] = make_runner(nc, N_CORES)
    return _cache[reps]


def kernel(x, Wq, Wk, Wv):
    r = get_runner(1)
    r.commit_inputs(make_inputs(x, Wq, Wk, Wv))
    return gather_output(r.results(r.run()))
